# revision 1
# baseline (speedup 1.0000x reference)
"""CoAttention forward on 8 TRN2 NeuronCores.

Data-parallel over batch B=64 (8 batches/core). Heavy matmuls run as f16
hi/lo multi-pass on the PE (3-pass ~22-bit mantissa, 1 cyc/row vs fp32's
4 cyc/row); the logit path (H storage, w_h dots, softmax) stays fp32.

Per batch b (Q [512,1024], V [196,1024], D=1024):
  A    = W_b^T Q^T               [D, NQ]   3-pass f16, split hi/lo on chip
  C    = tanh(A^T V^T)           [NQ, NV]  3-pass, stored f16
  CT   = C^T                     via PE f16 transposes
  WqQT = Q W_q^T                 [NQ, D]   3-pass, split hi/lo
  WvVT = V W_v^T                 [NV, D]   3-pass, split hi/lo
  G_v  = transpose(WvVT_hi) + (WqQT_hi + WqQT_lo) C     (f16 psum + f32 psum, DVE add)
  H_v  = tanh(G_v)  fp32;  h_v = w_hv^T H_v  fp32 -> softmax -> a_v
  G_q  = transpose(WqQT_hi) + (WvVT_hi + WvVT_lo) CT
  H_q  = tanh(G_q)  fp32;  h_q -> softmax -> a_q
  v_hat = sum_v a_v[v] VT_hi[:, v];  q_hat = sum_q a_q[q] QT_hi[:, q]

kernel(**inputs) takes FULL inputs, shards internally, returns (v_hat, q_hat).
"""
import numpy as np

import concourse.bass as bass
import concourse.mybir as mybir
import concourse.tile as tile
from concourse import bacc
from concourse.bass_utils import run_bass_kernel_spmd
from concourse.masks import make_identity

AF = mybir.ActivationFunctionType
ALU = mybir.AluOpType
AX = mybir.AxisListType
F32 = mybir.dt.float32
F16 = mybir.dt.float16

B, NV, NQ, D = 64, 196, 512, 1024
NCORES = 8
NB = B // NCORES          # batches per core
KD = D // 128             # 8 feature k-tiles
MQ = NQ // 128            # 4 NQ m-tiles
NV1 = NV - 128            # 68 (second NV tile)


def build(nb=NB):
    nc = bacc.Bacc(None, target_bir_lowering=False)

    QTh_d = nc.dram_tensor("QTh", [nb, D, NQ], F16, kind="ExternalInput")
    QTl_d = nc.dram_tensor("QTl", [nb, D, NQ], F16, kind="ExternalInput")
    VTh_d = nc.dram_tensor("VTh", [nb, D, NV], F16, kind="ExternalInput")
    VTl_d = nc.dram_tensor("VTl", [nb, D, NV], F16, kind="ExternalInput")
    Wbh_d = nc.dram_tensor("Wbh", [D, D], F16, kind="ExternalInput")
    Wbl_d = nc.dram_tensor("Wbl", [D, D], F16, kind="ExternalInput")
    WqTh_d = nc.dram_tensor("WqTh", [D, D], F16, kind="ExternalInput")
    WqTl_d = nc.dram_tensor("WqTl", [D, D], F16, kind="ExternalInput")
    WvTh_d = nc.dram_tensor("WvTh", [D, D], F16, kind="ExternalInput")
    WvTl_d = nc.dram_tensor("WvTl", [D, D], F16, kind="ExternalInput")
    whv_d = nc.dram_tensor("whv", [D, 1], F32, kind="ExternalInput")
    whq_d = nc.dram_tensor("whq", [D, 1], F32, kind="ExternalInput")
    OV_d = nc.dram_tensor("OV", [nb, D], F32, kind="ExternalOutput")
    OQ_d = nc.dram_tensor("OQ", [nb, D], F32, kind="ExternalOutput")

    with tile.TileContext(nc) as tc:
        with (
            tc.tile_pool(name="wsb", bufs=1) as wsb,
            tc.tile_pool(name="iop", bufs=2) as iop,
            tc.tile_pool(name="mid", bufs=1) as mid,
            tc.tile_pool(name="sm", bufs=1) as sm,
            tc.tile_pool(name="psp", bufs=4, space="PSUM") as psp,
        ):
            # ---- persistent weights ----
            def wtile(name, src):
                t = wsb.tile([128, KD, D], F16, name=name)
                nc.sync.dma_start(out=t, in_=src.rearrange("(k p) d -> p k d", p=128))
                return t

            wbh = wtile("wbh", Wbh_d)
            wbl = wtile("wbl", Wbl_d)
            wqth = wtile("wqth", WqTh_d)
            wqtl = wtile("wqtl", WqTl_d)
            wvth = wtile("wvth", WvTh_d)
            wvtl = wtile("wvtl", WvTl_d)
            whv_sb = wsb.tile([128, KD], F32)
            nc.sync.dma_start(out=whv_sb, in_=whv_d[:, 0].rearrange("(k p) -> p k", p=128))
            whq_sb = wsb.tile([128, KD], F32)
            nc.sync.dma_start(out=whq_sb, in_=whq_d[:, 0].rearrange("(k p) -> p k", p=128))
            identh = wsb.tile([128, 128], F16)
            make_identity(nc, identh)
            ones_row = wsb.tile([1, 128], F32)
            nc.vector.memset(ones_row, 1.0)

            for b in range(nb):
                qth = iop.tile([128, KD, NQ], F16, tag="qth")
                nc.sync.dma_start(out=qth, in_=QTh_d[b].rearrange("(k p) n -> p k n", p=128))
                qtl = iop.tile([128, KD, NQ], F16, tag="qtl")
                nc.sync.dma_start(out=qtl, in_=QTl_d[b].rearrange("(k p) n -> p k n", p=128))
                vth = iop.tile([128, KD, NV], F16, tag="vth")
                nc.sync.dma_start(out=vth, in_=VTh_d[b].rearrange("(k p) n -> p k n", p=128))
                vtl = iop.tile([128, KD, NV], F16, tag="vtl")
                nc.sync.dma_start(out=vtl, in_=VTl_d[b].rearrange("(k p) n -> p k n", p=128))

                # ---- phase 1+2 interleaved: A (3-pass), then C (3-pass) ----
                a_hi = mid.tile([128, KD, NQ], F16, tag="a_hi")
                a_lo = mid.tile([128, KD, NQ], F16, tag="a_lo")
                c_ps = [psp.tile([128, NV], F32, tag="ps196", name=f"c_ps{b}_{m}")
                        for m in range(MQ)]

                def emit_a(e):
                    pa = psp.tile([128, NQ], F32, tag="ps512", bufs=4, name=f"pa{b}_{e}")
                    passes = ((wbh, qth), (wbh, qtl), (wbl, qth))
                    es = slice(e * 128, (e + 1) * 128)
                    n = 0
                    for k in range(KD):
                        for lh, rh in passes:
                            n += 1
                            nc.tensor.matmul(pa, lh[:, k, es], rh[:, k, :],
                                             start=(n == 1), stop=(n == 3 * KD))
                    nc.vector.tensor_copy(a_hi[:, e, :], pa)
                    nc.vector.tensor_sub(a_lo[:, e, :], pa, a_hi[:, e, :])

                def emit_c(e):
                    for m in range(MQ):
                        ms = slice(m * 128, (m + 1) * 128)
                        for i, (lh, rh) in enumerate(((a_hi, vth), (a_hi, vtl), (a_lo, vth))):
                            nc.tensor.matmul(c_ps[m], lh[:, e, ms], rh[:, e, :],
                                             start=(e == 0 and i == 0),
                                             stop=(e == KD - 1 and i == 2))

                for e in range(KD + 1):
                    if e < KD:
                        emit_a(e)
                    if e >= 1:
                        emit_c(e - 1)

                c_sb = mid.tile([128, MQ, NV], F16, tag="c")
                for m in range(MQ):
                    nc.scalar.activation(c_sb[:, m, :], c_ps[m], AF.Tanh)

                # ---- CT via f16 PE transposes of C ----
                ct_sb = mid.tile([128, 2, NQ], F16, tag="ct")
                for mv in range(2):
                    rows = 128 if mv == 0 else NV1
                    ctp = psp.tile([128, NQ], F16, tag="ps512", bufs=4, name=f"ctp{b}_{mv}")
                    for mq in range(MQ):
                        nc.tensor.matmul(
                            ctp[:rows, mq * 128:(mq + 1) * 128],
                            c_sb[:, mq, mv * 128:mv * 128 + rows],
                            identh, is_transpose=True,
                            start=(mq == 0), stop=(mq == MQ - 1))
                    nc.scalar.copy(ct_sb[:rows, mv, :], ctp[:rows, :])

                # ---- phase 3: WqQT, WvVT (3-pass, split hi/lo) ----
                wqqt_hi = mid.tile([128, MQ, D], F16, tag="wqqt_hi")
                wqqt_lo = mid.tile([128, MQ, D], F16, tag="wqqt_lo")
                for m in range(MQ):
                    ms = slice(m * 128, (m + 1) * 128)
                    for h in range(2):
                        hs = slice(h * 512, (h + 1) * 512)
                        p = psp.tile([128, 512], F32, tag="ps512", bufs=4, name=f"pq{b}_{m}_{h}")
                        n = 0
                        for k in range(KD):
                            for lh, rh in ((qth, wqth), (qth, wqtl), (qtl, wqth)):
                                n += 1
                                nc.tensor.matmul(p, lh[:, k, ms], rh[:, k, hs],
                                                 start=(n == 1), stop=(n == 3 * KD))
                        nc.vector.tensor_copy(wqqt_hi[:, m, hs], p)
                        nc.vector.tensor_sub(wqqt_lo[:, m, hs], p, wqqt_hi[:, m, hs])
                wvvt_hi = mid.tile([128, 2, D], F16, tag="wvvt_hi")
                wvvt_lo = mid.tile([128, 2, D], F16, tag="wvvt_lo")
                for m in range(2):
                    rows = 128 if m == 0 else NV1
                    ms = slice(m * 128, m * 128 + rows)
                    for h in range(2):
                        hs = slice(h * 512, (h + 1) * 512)
                        p = psp.tile([128, 512], F32, tag="ps512", bufs=4, name=f"pv{b}_{m}_{h}")
                        n = 0
                        for k in range(KD):
                            for lh, rh in ((vth, wvth), (vth, wvtl), (vtl, wvth)):
                                n += 1
                                nc.tensor.matmul(p[:rows, :], lh[:, k, ms], rh[:, k, hs],
                                                 start=(n == 1), stop=(n == 3 * KD))
                        nc.vector.tensor_copy(wvvt_hi[:rows, m, hs], p[:rows, :])
                        nc.vector.tensor_sub(wvvt_lo[:rows, m, hs], p[:rows, :],
                                             wvvt_hi[:rows, m, hs])

                # ---- phase 4: H_v (f32) + h_v ----
                hv_m_l = [None] * KD
                h_v_ps = psp.tile([1, NV], F32, tag="ps196", name=f"hv_acc{b}")

                def emit_hv(m):
                    ms = slice(m * 128, (m + 1) * 128)
                    t2 = psp.tile([128, NV], F32, tag="ps196", name=f"hv2_{b}_{m}")
                    for kq in range(MQ):
                        for i, lh in enumerate((wqqt_hi, wqqt_lo)):
                            nc.tensor.matmul(t2, lh[:, kq, ms], c_sb[:, kq, :],
                                             start=(kq == 0 and i == 0),
                                             stop=(kq == MQ - 1 and i == 1))
                    t1sb = [None, None]
                    for i, w in enumerate((wvvt_hi, wvvt_lo)):
                        t1 = psp.tile([128, NV], F16, tag="ps196",
                                      name=f"hv1_{b}_{m}_{i}")
                        nc.tensor.matmul(t1[:, 0:128], w[:, 0, ms], identh,
                                         is_transpose=True, start=True, stop=False)
                        nc.tensor.matmul(t1[:, 128:NV], w[:NV1, 1, ms],
                                         identh[:NV1, :NV1],
                                         is_transpose=True, start=False, stop=True)
                        t1sb[i] = sm.tile([128, NV], F16, tag=f"t1v{i}", bufs=2,
                                          name=f"t1v{b}_{m}_{i}")
                        nc.scalar.copy(t1sb[i], t1)
                    pre = sm.tile([128, NV], F32, tag="prev", bufs=1, name=f"prev{b}_{m}")
                    nc.vector.scalar_tensor_tensor(out=pre, in0=t2, scalar=1.0, in1=t1sb[0],
                                                   op0=ALU.mult, op1=ALU.add)
                    nc.vector.tensor_add(pre, pre, t1sb[1])
                    hv_m = sm.tile([128, NV], F32, tag="hvm", bufs=2, name=f"hvm{b}_{m}")
                    nc.scalar.activation(hv_m, pre, AF.Tanh)
                    hv_m_l[m] = hv_m

                def emit_hv_dot(m):
                    nc.tensor.matmul(h_v_ps, whv_sb[:, m:m + 1], hv_m_l[m],
                                     start=(m == 0), stop=(m == KD - 1))

                for m in range(KD + 1):
                    if m < KD:
                        emit_hv(m)
                    if m >= 1:
                        emit_hv_dot(m - 1)

                # ---- phase 5: H_q (f32) + h_q ----
                hq_m_l = [None] * KD
                h_q_ps = psp.tile([1, NQ], F32, tag="ps512", bufs=4, name=f"hq_acc{b}")

                def emit_hq(m):
                    ms = slice(m * 128, (m + 1) * 128)
                    t2 = psp.tile([128, NQ], F32, tag="ps512", bufs=4, name=f"hq2_{b}_{m}")
                    for kv in range(2):
                        rows = 128 if kv == 0 else NV1
                        for i, lh in enumerate((wvvt_hi, wvvt_lo)):
                            nc.tensor.matmul(t2, lh[:rows, kv, ms], ct_sb[:rows, kv, :],
                                             start=(kv == 0 and i == 0),
                                             stop=(kv == 1 and i == 1))
                    t1sb = [None, None]
                    for i, w in enumerate((wqqt_hi, wqqt_lo)):
                        t1 = psp.tile([128, NQ], F16, tag="ps512", bufs=4,
                                      name=f"hq1_{b}_{m}_{i}")
                        for kq in range(MQ):
                            nc.tensor.matmul(t1[:, kq * 128:(kq + 1) * 128],
                                             w[:, kq, ms], identh, is_transpose=True,
                                             start=(kq == 0), stop=(kq == MQ - 1))
                        t1sb[i] = sm.tile([128, NQ], F16, tag=f"t1q{i}", bufs=2,
                                          name=f"t1q{b}_{m}_{i}")
                        nc.scalar.copy(t1sb[i], t1)
                    pre = sm.tile([128, NQ], F32, tag="preq", bufs=1, name=f"preq{b}_{m}")
                    nc.vector.scalar_tensor_tensor(out=pre, in0=t2, scalar=1.0, in1=t1sb[0],
                                                   op0=ALU.mult, op1=ALU.add)
                    nc.vector.tensor_add(pre, pre, t1sb[1])
                    hq_m = sm.tile([128, NQ], F32, tag="hqm", bufs=2, name=f"hqm{b}_{m}")
                    nc.scalar.activation(hq_m, pre, AF.Tanh)
                    hq_m_l[m] = hq_m

                def emit_hq_dot(m):
                    nc.tensor.matmul(h_q_ps, whq_sb[:, m:m + 1], hq_m_l[m],
                                     start=(m == 0), stop=(m == KD - 1))

                for m in range(KD + 1):
                    if m < KD:
                        emit_hq(m)
                    if m >= 1:
                        emit_hq_dot(m - 1)

                # ---- phase 6: softmaxes + on-chip broadcast ----
                def softmax_bcast(h_ps, n, tagp):
                    negm = sm.tile([1, 1], F32, tag=f"negm{tagp}")
                    nc.vector.reduce_max(negm, h_ps, axis=AX.X, negate=True)
                    ex = sm.tile([1, n], F32, tag=f"ex{tagp}")
                    ssum = sm.tile([1, 1], F32, tag=f"ssum{tagp}")
                    nc.scalar.activation(ex, h_ps, AF.Exp, bias=negm, accum_out=ssum)
                    rs = sm.tile([1, 1], F32, tag=f"rs{tagp}")
                    nc.vector.reciprocal(rs, ssum)
                    ones_s = sm.tile([1, 128], F32, tag=f"ones_s{tagp}")
                    nc.vector.tensor_scalar_mul(ones_s, ones_row, rs)
                    ab_ps = psp.tile([128, n], F32, tag="ps512", bufs=4, name=f"abps{tagp}{b}")
                    nc.tensor.matmul(ab_ps, ones_s, ex, start=True, stop=True)
                    ab = sm.tile([128, n], F32, tag=f"ab{tagp}")
                    nc.scalar.copy(ab, ab_ps)
                    return ab

                av_b = softmax_bcast(h_v_ps, NV, "v")
                aq_b = softmax_bcast(h_q_ps, NQ, "q")

                # ---- phase 7: v_hat / q_hat (hi+lo two-pass) ----
                vhat_sb = sm.tile([128, KD], F32, tag="vhat")
                vhat2_sb = sm.tile([128, KD], F32, tag="vhat2")
                scrv = sm.tile([128, NV], F16, tag="scrv")
                for k in range(KD):
                    nc.vector.scalar_tensor_tensor(
                        out=scrv, in0=vth[:, k, :], scalar=1.0, in1=av_b,
                        op0=ALU.mult, op1=ALU.mult, accum_out=vhat_sb[:, k:k + 1])
                    nc.vector.scalar_tensor_tensor(
                        out=scrv, in0=vtl[:, k, :], scalar=1.0, in1=av_b,
                        op0=ALU.mult, op1=ALU.mult, accum_out=vhat2_sb[:, k:k + 1])
                nc.vector.tensor_add(vhat_sb, vhat_sb, vhat2_sb)
                qhat_sb = sm.tile([128, KD], F32, tag="qhat")
                qhat2_sb = sm.tile([128, KD], F32, tag="qhat2")
                scrq = sm.tile([128, NQ], F16, tag="scrq")
                for k in range(KD):
                    nc.vector.scalar_tensor_tensor(
                        out=scrq, in0=qth[:, k, :], scalar=1.0, in1=aq_b,
                        op0=ALU.mult, op1=ALU.mult, accum_out=qhat_sb[:, k:k + 1])
                    nc.vector.scalar_tensor_tensor(
                        out=scrq, in0=qtl[:, k, :], scalar=1.0, in1=aq_b,
                        op0=ALU.mult, op1=ALU.mult, accum_out=qhat2_sb[:, k:k + 1])
                nc.vector.tensor_add(qhat_sb, qhat_sb, qhat2_sb)
                nc.sync.dma_start(out=OV_d[b].rearrange("(k p) -> p k", p=128), in_=vhat_sb)
                nc.sync.dma_start(out=OQ_d[b].rearrange("(k p) -> p k", p=128), in_=qhat_sb)

    nc.finalize()
    return nc


_BUILT = {}


def _split(x):
    hi = x.astype(np.float16)
    lo = (x - hi.astype(np.float32)).astype(np.float16)
    return np.ascontiguousarray(hi), np.ascontiguousarray(lo)


def kernel(V, Q, W_b, W_v, W_q, w_hv, w_hq, _trace=False):
    V = np.asarray(V, dtype=np.float32)
    Q = np.asarray(Q, dtype=np.float32)
    nb = B // NCORES
    QTh, QTl = _split(Q.transpose(0, 2, 1))      # [B, D, NQ] f16
    VTh, VTl = _split(V.transpose(0, 2, 1))      # [B, D, NV] f16
    Wbh, Wbl = _split(np.asarray(W_b, dtype=np.float32))
    WqTh, WqTl = _split(np.asarray(W_q, dtype=np.float32).T)
    WvTh, WvTl = _split(np.asarray(W_v, dtype=np.float32).T)
    whv = np.ascontiguousarray(np.asarray(w_hv, dtype=np.float32))
    whq = np.ascontiguousarray(np.asarray(w_hq, dtype=np.float32))

    if nb not in _BUILT:
        _BUILT[nb] = build(nb)
    nc = _BUILT[nb]

    in_maps = []
    for c in range(NCORES):
        sl = slice(c * nb, (c + 1) * nb)
        in_maps.append({
            "QTh": np.ascontiguousarray(QTh[sl]), "QTl": np.ascontiguousarray(QTl[sl]),
            "VTh": np.ascontiguousarray(VTh[sl]), "VTl": np.ascontiguousarray(VTl[sl]),
            "Wbh": Wbh, "Wbl": Wbl, "WqTh": WqTh, "WqTl": WqTl,
            "WvTh": WvTh, "WvTl": WvTl, "whv": whv, "whq": whq,
        })

    out = run_bass_kernel_spmd(nc, in_maps, core_ids=list(range(NCORES)),
                               trace=_trace)
    v_hat = np.concatenate([out.results[c]["OV"] for c in range(NCORES)], axis=0)
    q_hat = np.concatenate([out.results[c]["OQ"] for c in range(NCORES)], axis=0)
    if _trace:
        kernel._last_exec_ns = out.exec_time_ns
        kernel._last_results = out
    return (v_hat, q_hat)



# revision 11
# speedup vs baseline: 1.4234x; 1.4234x over previous
"""CoAttention forward on 8 TRN2 NeuronCores — layout-B restructure.

Data-parallel over batch B=64 (8 batches/core). All heavy products run as
f16 3-pass (hh + h*lo + lo*h ~ 22-bit values) with hi/lo f16 storage of
intermediates; G_v/G_q assemble their direct term exactly in f32 PSUM, so
only the cross terms (t2', S) pay 2-pass hi/lo cost. Logits stay f32.

Per batch b (Q [512,1024], V [196,1024], D=1024):
  U    = W_b V^T                [D(e), NV]  3-pass, stored hi/lo f16
  C    = tanh(Q U)              [NQ, NV]    3-pass, stored f16 (tanh-saturated)
  CT   = C^T                    PE f16 transposes
  per d-half (512):
    WvVT = V W_v^T              [NV, d]     3-pass -> psum kept + hi/lo sbuf
    per q-chunk: G_q^T = C WvVT(2-pass hi/lo) ++ Q W_q^T(3-pass, psum)
                 -> wqqt hi/lo sbuf, H_q = tanh f16
    G_v^T = WvVT(psum) ++ C^T wqqt (2-pass)  -> H_v = tanh f16
  h_v/h_q: DVE dot (H f16 * w-broadcast f16, accum f32) -> PE f32 col
  transpose -> softmax f32 -> a broadcast via PE -> v_hat/q_hat DVE STT.
"""
import numpy as np

import concourse.bass as bass
import concourse.mybir as mybir
import concourse.tile as tile
from concourse import bacc
from concourse.bass_utils import run_bass_kernel_spmd
from concourse.masks import make_identity

AF = mybir.ActivationFunctionType
ALU = mybir.AluOpType
AX = mybir.AxisListType
F32 = mybir.dt.float32
F16 = mybir.dt.float16

B, NV, NQ, D = 64, 196, 512, 1024
NCORES = 8
NB = B // NCORES
KD = D // 128             # 8 feature k-chunks
MQ = NQ // 128            # 4 q-chunks
NV1 = NV - 128            # 68 rows in second v-chunk
VROWS = (128, NV1)


def build(nb=NB):
    nc = bacc.Bacc(None, target_bir_lowering=False)

    QTh_d = nc.dram_tensor("QTh", [nb, D, NQ], F16, kind="ExternalInput")
    QTl_d = nc.dram_tensor("QTl", [nb, D, NQ], F16, kind="ExternalInput")
    VTh_d = nc.dram_tensor("VTh", [nb, D, NV], F16, kind="ExternalInput")
    VTl_d = nc.dram_tensor("VTl", [nb, D, NV], F16, kind="ExternalInput")
    WbTh_d = nc.dram_tensor("WbTh", [D, D], F16, kind="ExternalInput")
    WbTl_d = nc.dram_tensor("WbTl", [D, D], F16, kind="ExternalInput")
    WqTh_d = nc.dram_tensor("WqTh", [D, D], F16, kind="ExternalInput")
    WqTl_d = nc.dram_tensor("WqTl", [D, D], F16, kind="ExternalInput")
    WvTh_d = nc.dram_tensor("WvTh", [D, D], F16, kind="ExternalInput")
    WvTl_d = nc.dram_tensor("WvTl", [D, D], F16, kind="ExternalInput")
    whv_d = nc.dram_tensor("whv", [1, D], F16, kind="ExternalInput")
    whq_d = nc.dram_tensor("whq", [1, D], F16, kind="ExternalInput")
    OV_d = nc.dram_tensor("OV", [nb, D], F32, kind="ExternalOutput")
    OQ_d = nc.dram_tensor("OQ", [nb, D], F32, kind="ExternalOutput")

    with tile.TileContext(nc) as tc:
        with (
            tc.tile_pool(name="wsb", bufs=1) as wsb,
            tc.tile_pool(name="iop", bufs=2) as iop,
            tc.tile_pool(name="mid", bufs=1) as mid,
            tc.tile_pool(name="sm", bufs=1) as sm,
            tc.tile_pool(name="psp", bufs=1, space="PSUM") as psp,
        ):
            # ---- persistent weights ----
            def wtile(name, src):
                t = wsb.tile([128, KD, D], F16, name=name)
                nc.sync.dma_start(out=t, in_=src.rearrange("(k p) d -> p k d", p=128))
                return t

            wbth = wtile("wbth", WbTh_d)
            wbtl = wtile("wbtl", WbTl_d)
            wqth = wtile("wqth", WqTh_d)
            wqtl = wtile("wqtl", WqTl_d)
            wvth = wtile("wvth", WvTh_d)
            wvtl = wtile("wvtl", WvTl_d)
            identh = wsb.tile([128, 128], F16)
            make_identity(nc, identh)
            identf = wsb.tile([128, 128], F32)
            make_identity(nc, identf)
            ones16 = wsb.tile([1, 128], F16)
            nc.vector.memset(ones16, 1.0)

            # broadcast w_hv / w_hq rows to [128, D] f16
            whv_r16 = wsb.tile([1, D], F16)
            nc.sync.dma_start(out=whv_r16, in_=whv_d[:, :])
            whq_r16 = wsb.tile([1, D], F16)
            nc.sync.dma_start(out=whq_r16, in_=whq_d[:, :])
            whv_b = wsb.tile([128, D], F16)
            whq_b = wsb.tile([128, D], F16)
            for h in range(2):
                hs = slice(h * 512, (h + 1) * 512)
                for bt, row in ((whv_b, whv_r16), (whq_b, whq_r16)):
                    pb = psp.tile([128, 512], F32, tag="puc", bufs=3,
                                  name=f"pbw{h}_{0 if bt is whv_b else 1}")
                    nc.tensor.matmul(pb, ones16, row[:, hs], start=True, stop=True)
                    nc.scalar.copy(bt[:, hs], pb)

            for b in range(nb):
                qth = iop.tile([128, KD, NQ], F16, tag="qth")
                nc.sync.dma_start(out=qth, in_=QTh_d[b].rearrange("(k p) n -> p k n", p=128))
                qtl = iop.tile([128, KD, NQ], F16, tag="qtl")
                nc.sync.dma_start(out=qtl, in_=QTl_d[b].rearrange("(k p) n -> p k n", p=128))
                vth = iop.tile([128, KD, NV], F16, tag="vth")
                nc.sync.dma_start(out=vth, in_=VTh_d[b].rearrange("(k p) n -> p k n", p=128))
                vtl = iop.tile([128, KD, NV], F16, tag="vtl")
                nc.sync.dma_start(out=vtl, in_=VTl_d[b].rearrange("(k p) n -> p k n", p=128))

                # ---- U = W_b V^T  [e, v], 3-pass, hi/lo ----
                u_h = mid.tile([128, KD, NV], F16, tag="u_h")
                u_l = mid.tile([128, KD, NV], F16, tag="u_l")
                for e in range(KD):
                    es = slice(e * 128, (e + 1) * 128)
                    pu = psp.tile([128, 512], F32, tag="puc", bufs=3, name=f"pu{b}_{e}")
                    n = 0
                    for k in range(KD):
                        for lh, rh in ((wbth, vth), (wbth, vtl), (wbtl, vth)):
                            n += 1
                            nc.tensor.matmul(pu[:, :NV], lh[:, k, es], rh[:, k, :],
                                             start=(n == 1), stop=(n == 3 * KD))
                    nc.scalar.copy(u_h[:, e, :], pu[:, :NV])
                    nc.vector.tensor_sub(u_l[:, e, :], pu[:, :NV], u_h[:, e, :])

                # ---- C = tanh(Q U)  [q, v], 3-pass, f16 ----
                c16 = mid.tile([128, MQ, NV], F16, tag="c16")
                for m in range(MQ):
                    ms = slice(m * 128, (m + 1) * 128)
                    pc = psp.tile([128, 512], F32, tag="puc", bufs=3, name=f"pc{b}_{m}")
                    n = 0
                    for e in range(KD):
                        for lh, rh in ((qth, u_h), (qth, u_l), (qtl, u_h)):
                            n += 1
                            nc.tensor.matmul(pc[:, :NV], lh[:, e, ms], rh[:, e, :],
                                             start=(n == 1), stop=(n == 3 * KD))
                    nc.scalar.activation(c16[:, m, :], pc[:, :NV], AF.Tanh)

                # ---- CT = C^T  [v, q] f16 via 128x128 PE transposes ----
                ct16 = mid.tile([128, 2, NQ], F16, tag="ct16")
                for mv in range(2):
                    rows = VROWS[mv]
                    vs = slice(mv * 128, mv * 128 + rows)
                    for mq in range(MQ):
                        pt = psp.tile([128, 128], F16, tag="pcts", bufs=1,
                                      name=f"pt{b}_{mv}_{mq}")
                        nc.tensor.transpose(pt[:rows, :], c16[:, mq, vs], identh)
                        nc.scalar.copy(ct16[:rows, mv, mq * 128:(mq + 1) * 128],
                                       pt[:rows, :])

                # ---- per d-half: WvVT, G_q^T, G_v^T ----
                wvvt_h = mid.tile([128, 2, D], F16, tag="wvvt_h")
                wvvt_l = mid.tile([128, 2, D], F16, tag="wvvt_l")
                wqqt_h = mid.tile([128, MQ, D], F16, tag="wqqt_h")
                wqqt_l = mid.tile([128, MQ, D], F16, tag="wqqt_l")
                hv16 = mid.tile([128, 2, D], F16, tag="hv16")
                hq16 = mid.tile([128, MQ, D], F16, tag="hq16")
                for h in range(2):
                    hs = slice(h * 512, (h + 1) * 512)
                    # (a) WvVT chunks into pv psum (kept open for (c))
                    pv_t = []
                    for mv in range(2):
                        rows = VROWS[mv]
                        vs = slice(mv * 128, mv * 128 + rows)
                        pvt = psp.tile([128, 512], F32, tag="pv", bufs=2,
                                       name=f"pv{b}_{h}_{mv}")
                        n = 0
                        for k in range(KD):
                            for lh, rh in ((vth, wvth), (vth, wvtl), (vtl, wvth)):
                                n += 1
                                nc.tensor.matmul(pvt[:rows, :], lh[:, k, vs],
                                                 rh[:, k, hs],
                                                 start=(n == 1), stop=False)
                        nc.scalar.copy(wvvt_h[:rows, mv, hs], pvt[:rows, :])
                        nc.vector.tensor_sub(wvvt_l[:rows, mv, hs], pvt[:rows, :],
                                             wvvt_h[:rows, mv, hs])
                        pv_t.append(pvt)
                    # (b) per q-chunk: G_q^T = S(2-pass) + WqQT(3-pass)
                    for mq in range(MQ):
                        ms = slice(mq * 128, (mq + 1) * 128)
                        pqt = psp.tile([128, 512], F32, tag="pq", bufs=2,
                                       name=f"pq{b}_{h}_{mq}")
                        nm = 0
                        for k in range(KD):
                            for lh, rh in ((qth, wqth), (qth, wqtl), (qtl, wqth)):
                                nm += 1
                                nc.tensor.matmul(pqt, lh[:, k, ms], rh[:, k, hs],
                                                 start=(nm == 1), stop=False)
                        # snapshot WqQT (hi/lo) before S accumulates on top
                        nc.scalar.copy(wqqt_h[:, mq, hs], pqt)
                        nc.vector.tensor_sub(wqqt_l[:, mq, hs], pqt,
                                             wqqt_h[:, mq, hs])
                        n = 0
                        for mv in range(2):
                            rows = VROWS[mv]
                            for rh in (wvvt_h, wvvt_l):
                                n += 1
                                nc.tensor.matmul(pqt, ct16[:rows, mv, ms],
                                                 rh[:rows, mv, hs],
                                                 start=False, stop=(n == 4))
                        nc.scalar.activation(hq16[:, mq, hs], pqt, AF.Tanh)
                    # (c) G_v^T: t2' accumulates onto WvVT psum
                    for mv in range(2):
                        rows = VROWS[mv]
                        vs = slice(mv * 128, mv * 128 + rows)
                        n = 0
                        for mq in range(MQ):
                            for rh in (wqqt_h, wqqt_l):
                                n += 1
                                nc.tensor.matmul(pv_t[mv][:rows, :],
                                                 c16[:, mq, vs], rh[:, mq, hs],
                                                 start=False, stop=(n == 2 * MQ))
                        nc.scalar.activation(hv16[:rows, mv, hs],
                                             pv_t[mv][:rows, :], AF.Tanh)

                # ---- logits: DVE dot + PE f32 col transpose ----
                hvc = mid.tile([128, 2], F32, tag="hvc")
                hqc = mid.tile([128, MQ], F32, tag="hqc")
                scr = mid.tile([128, D], F16, tag="scr")
                for mv in range(2):
                    rows = VROWS[mv]
                    nc.vector.scalar_tensor_tensor(
                        out=scr[:rows, :], in0=hv16[:rows, mv, :], scalar=1.0,
                        in1=whv_b[:rows, :], op0=ALU.mult, op1=ALU.mult,
                        accum_out=hvc[:rows, mv:mv + 1])
                for mq in range(MQ):
                    nc.vector.scalar_tensor_tensor(
                        out=scr, in0=hq16[:, mq, :], scalar=1.0,
                        in1=whq_b, op0=ALU.mult, op1=ALU.mult,
                        accum_out=hqc[:, mq:mq + 1])

                hps_v = psp.tile([128, 512], F32, tag="puc", bufs=3, name=f"hpv{b}")
                for mv in range(2):
                    rows = VROWS[mv]
                    nc.tensor.transpose(hps_v[0:1, mv * 128:mv * 128 + rows],
                                        hvc[:rows, mv:mv + 1], identf[:rows, :rows])
                hps_q = psp.tile([128, 512], F32, tag="puc", bufs=3, name=f"hpq{b}")
                for mq in range(MQ):
                    nc.tensor.transpose(hps_q[0:1, mq * 128:(mq + 1) * 128],
                                        hqc[:, mq:mq + 1], identf)

                # ---- softmax + broadcast ----
                def softmax_bcast(h_ps, n, tagp):
                    negm = sm.tile([1, 1], F32, tag=f"negm{tagp}")
                    nc.vector.reduce_max(negm, h_ps[0:1, :n], axis=AX.X, negate=True)
                    ex = sm.tile([1, n], F32, tag=f"ex{tagp}")
                    ssum = sm.tile([1, 1], F32, tag=f"ssum{tagp}")
                    nc.scalar.activation(ex, h_ps[0:1, :n], AF.Exp, bias=negm,
                                         accum_out=ssum)
                    rs = sm.tile([1, 1], F32, tag=f"rs{tagp}")
                    nc.vector.reciprocal(rs, ssum)
                    ex16 = sm.tile([1, n], F16, tag=f"ex16{tagp}")
                    nc.scalar.mul(ex16, ex, rs)
                    ab_ps = psp.tile([128, 512], F32, tag="puc", bufs=3,
                                     name=f"abps{tagp}{b}")
                    nc.tensor.matmul(ab_ps[:, :n], ones16, ex16, start=True, stop=True)
                    return ab_ps

                av_b = softmax_bcast(hps_v, NV, "v")
                aq_b = softmax_bcast(hps_q, NQ, "q")

                # ---- v_hat / q_hat ----
                vhat_sb = sm.tile([128, KD], F32, tag="vhat")
                qhat_sb = sm.tile([128, KD], F32, tag="qhat")
                for k in range(KD):
                    nc.vector.scalar_tensor_tensor(
                        out=scr[:, :NV], in0=vth[:, k, :], scalar=1.0,
                        in1=av_b[:, :NV], op0=ALU.mult, op1=ALU.mult,
                        accum_out=vhat_sb[:, k:k + 1])
                for k in range(KD):
                    nc.vector.scalar_tensor_tensor(
                        out=scr[:, :NQ], in0=qth[:, k, :], scalar=1.0,
                        in1=aq_b[:, :NQ], op0=ALU.mult, op1=ALU.mult,
                        accum_out=qhat_sb[:, k:k + 1])
                nc.sync.dma_start(out=OV_d[b].rearrange("(k p) -> p k", p=128), in_=vhat_sb)
                nc.sync.dma_start(out=OQ_d[b].rearrange("(k p) -> p k", p=128), in_=qhat_sb)

    nc.finalize()
    return nc


_BUILT = {}


def _split(x):
    hi = x.astype(np.float16)
    lo = (x - hi.astype(np.float32)).astype(np.float16)
    return np.ascontiguousarray(hi), np.ascontiguousarray(lo)


def kernel(V, Q, W_b, W_v, W_q, w_hv, w_hq, _trace=False):
    V = np.asarray(V, dtype=np.float32)
    Q = np.asarray(Q, dtype=np.float32)
    nb = B // NCORES
    QTh, QTl = _split(Q.transpose(0, 2, 1))      # [B, D, NQ] f16
    VTh, VTl = _split(V.transpose(0, 2, 1))      # [B, D, NV] f16
    WbTh, WbTl = _split(np.asarray(W_b, dtype=np.float32).T)
    WqTh, WqTl = _split(np.asarray(W_q, dtype=np.float32).T)
    WvTh, WvTl = _split(np.asarray(W_v, dtype=np.float32).T)
    whv = np.ascontiguousarray(np.asarray(w_hv, dtype=np.float32).reshape(1, D).astype(np.float16))
    whq = np.ascontiguousarray(np.asarray(w_hq, dtype=np.float32).reshape(1, D).astype(np.float16))

    if nb not in _BUILT:
        _BUILT[nb] = build(nb)
    nc = _BUILT[nb]

    in_maps = []
    for c in range(NCORES):
        sl = slice(c * nb, (c + 1) * nb)
        in_maps.append({
            "QTh": np.ascontiguousarray(QTh[sl]), "QTl": np.ascontiguousarray(QTl[sl]),
            "VTh": np.ascontiguousarray(VTh[sl]), "VTl": np.ascontiguousarray(VTl[sl]),
            "WbTh": WbTh, "WbTl": WbTl, "WqTh": WqTh, "WqTl": WqTl,
            "WvTh": WvTh, "WvTl": WvTl, "whv": whv, "whq": whq,
        })

    out = run_bass_kernel_spmd(nc, in_maps, core_ids=list(range(NCORES)),
                               trace=_trace)
    v_hat = np.concatenate([out.results[c]["OV"] for c in range(NCORES)], axis=0)
    q_hat = np.concatenate([out.results[c]["OQ"] for c in range(NCORES)], axis=0)
    if _trace:
        kernel._last_exec_ns = out.exec_time_ns
        kernel._last_results = out
    return (v_hat, q_hat)
